# revision 9
# baseline (speedup 1.0000x reference)
"""CascadeNNBN Trainium2 kernel.

8-way data-parallel over the batch dim. Each core holds a 2048-row shard
of the batch with features kept TRANSPOSED in SBUF (features on
partitions, batch on the free axis), so every cascade matmul contracts
over the partition dim with no on-device transposes:

    h_i^T [256, 2048] = W_i @ feats^T   (lhsT = W_i^T, host-pretransposed)

BatchNorm batch statistics are raw per-core (sum, sum-of-squares) pairs,
exchanged with one tiny AllGather per stage and summed locally. A
warm-up AllGather at t=0 absorbs the one-time collective bootstrap /
barrier cost (~55us) so the per-stage exchanges run at the ~5us
steady-state latency.

Schedule: stage i+1's AR-independent k-tiles and two output-matmul
k-tiles fill each stage-i collective window. Only window 0 (the long
bootstrap barrier) uses parking: the x-block partial sums of stages 2-7
are pre-accumulated there, copied PSUM->SBUF on ScalarE (keeping
VectorE free for the latency-critical BN chain), and re-injected later
via an exact identity matmul. Normalization is split ScalarE/VectorE
per half so the gated matmuls start ~0.5us after the BN coefficients.

Matmuls run in bf16 (fp32 PSUM accumulation); statistics, normalization
coefficients and the final output are fp32.
"""

import sys

if "/opt/trn_rl_repo" not in sys.path:
    sys.path.insert(0, "/opt/trn_rl_repo")

import numpy as np
from ml_dtypes import bfloat16

import concourse.bass as bass  # noqa: F401  (import keeps bass registered)
import concourse.mybir as mybir
import concourse.tile as tile
from concourse import bacc
from concourse.bass_utils import run_bass_kernel_spmd
from concourse.masks import make_identity

N_CORES = 8
B = 16384
BSH = B // N_CORES          # 2048 batch rows per core
DIN = 512
K = 8                       # cascade stages
WS = 256                    # neurons per stage
DOUT = 128
EPS = 1e-5
P = 128
NB = BSH // 512             # batch chunks of 512 (PSUM bank free dim)
KO = [(DIN + WS * i) // P for i in range(K)]   # k-tiles per stage: 4,6,...,18
T_TOT = (DIN + WS * K) // P                    # 20 F tiles
PARKED = (2, 3, 4, 5, 6, 7)  # stages whose x-part is parked in window 0

_NC_CACHE = {}

# test-harness knobs (ignored in normal use): when TRACE_DIR is set the
# device run is profiled and kernel() stores the BassKernelResults here.
TRACE_DIR = None
LAST_RESULTS = None

BF = mybir.dt.bfloat16
F32 = mybir.dt.float32


def _build_nc():
    nc = bacc.Bacc("TRN2", target_bir_lowering=False, debug=False,
                   num_devices=N_CORES)

    xt_d = nc.dram_tensor("xt", [P, DIN // P, BSH], BF, kind="ExternalInput")
    w_d = [
        nc.dram_tensor(f"w{i}", [P, KO[i], WS], BF, kind="ExternalInput")
        for i in range(K)
    ]
    wo_d = nc.dram_tensor("wo", [P, T_TOT, DOUT], BF, kind="ExternalInput")
    bv_d = nc.dram_tensor("bv", [P, K, 2], F32, kind="ExternalInput")
    gv_d = nc.dram_tensor("gv", [P, K, 2], F32, kind="ExternalInput")
    bev_d = nc.dram_tensor("bev", [P, K, 2], F32, kind="ExternalInput")
    bout_d = nc.dram_tensor("boutv", [P, 1], F32, kind="ExternalInput")
    outT_d = nc.dram_tensor("outT", [P, BSH], F32, kind="ExternalOutput")

    with tile.TileContext(nc) as tc:
        _emit(nc, tc, xt_d, w_d, wo_d, bv_d, gv_d, bev_d, bout_d, outT_d)
    nc.compile()
    return nc


def _emit(nc, tc, xt_d, w_d, wo_d, bv_d, gv_d, bev_d, bout_d, outT_d):
    AF = mybir.ActivationFunctionType
    OP = mybir.AluOpType
    groups = [list(range(N_CORES))]

    with (
        tc.tile_pool(name="big", bufs=1) as big,
        tc.tile_pool(name="hp", bufs=1) as hp,
        tc.tile_pool(name="scrp", bufs=3) as scrp,
        tc.tile_pool(name="small", bufs=2) as small,
        tc.tile_pool(name="ps", bufs=8, space="PSUM") as ps,
        tc.tile_pool(name="dram", bufs=2, space="DRAM") as dram,
    ):
        # ---- persistent SBUF ----
        F = [big.tile([P, BSH], BF, tag=f"F{t}", name=f"F{t}") for t in range(T_TOT)]
        Wsb = [big.tile([P, KO[i], WS], BF, tag=f"W{i}", name=f"W{i}") for i in range(K)]
        WO = big.tile([P, T_TOT, DOUT], BF, tag="WO")
        BV = big.tile([P, K, 2], F32, tag="BV")
        GV = big.tile([P, K, 2], F32, tag="GV")
        BEV = big.tile([P, K, 2], F32, tag="BEV")
        BOUT = big.tile([P, 1], F32, tag="BOUT")
        OUTACC = big.tile([P, BSH], F32, tag="OUTACC")
        EPSC = big.tile([P, 1], F32, tag="EPSC")
        nc.vector.memset(EPSC[:], EPS)
        # identity (bf16) for re-injecting SBUF partial sums into PSUM
        IDT = big.tile([P, P], BF, tag="IDT")
        make_identity(nc, IDT[:, :])
        # x-block partial sums of stages 2..7, parked during window 0
        HACC = {j: big.tile([P, 2, BSH], BF, tag=f"HACC{j}", name=f"HACC{j}")
                for j in PARKED}

        # ---- warm-up collective: absorbs the one-time barrier/bootstrap
        # cost (~55us observed) so the first real AllGather runs at the
        # steady-state ~5us latency. Triggered at t=0, result unused.
        warm_in = dram.tile([P, 2], F32, tag="warm_in")
        warm_out = dram.tile([N_CORES, P, 2], F32, tag="warm_out",
                             addr_space="Shared")
        nc.gpsimd.collective_compute(
            "AllGather", OP.bypass, replica_groups=groups,
            ins=[warm_in.opt()], outs=[warm_out.opt()],
        )

        # ---- input DMAs: three queues, ordered by first consumption ----
        for ko in range(KO[0]):
            nc.gpsimd.dma_start(Wsb[0][:, ko, :], w_d[0][:, ko, :])
        nc.gpsimd.dma_start(WO[:], wo_d[:, :, :])
        nc.gpsimd.dma_start(BV[:], bv_d[:, :, :])
        nc.gpsimd.dma_start(GV[:], gv_d[:, :, :])
        nc.gpsimd.dma_start(BEV[:], bev_d[:, :, :])
        nc.gpsimd.dma_start(BOUT[:], bout_d[:, :])
        nc.gpsimd.dma_start(Wsb[3][:], w_d[3][:, :, :])
        nc.gpsimd.dma_start(Wsb[7][:], w_d[7][:, :, :])
        for t in range(DIN // P):
            eng = nc.sync if t % 2 == 0 else nc.scalar
            for bb in range(NB):
                eng.dma_start(F[t][:, bb * 512:(bb + 1) * 512],
                              xt_d[:, t, bb * 512:(bb + 1) * 512])
        nc.sync.dma_start(Wsb[5][:], w_d[5][:, :, :])
        nc.sync.dma_start(Wsb[6][:], w_d[6][:, :, :])
        nc.scalar.dma_start(Wsb[1][:], w_d[1][:, :, :])
        nc.scalar.dma_start(Wsb[2][:], w_d[2][:, :, :])
        nc.scalar.dma_start(Wsb[4][:], w_d[4][:, :, :])

        def mm(pt, lhsT, k, bb, start, stop):
            nc.tensor.matmul(
                pt[:, :], lhsT, F[k][:, bb * 512:(bb + 1) * 512],
                start=start, stop=stop)

        def alloc_group():
            return [
                [ps.tile([P, 512], F32, tag="pt", name="pt") for _ in range(NB)]
                for _ in range(2)
            ]

        def park_group(j):
            """Accumulate stage j's x k-tiles; park in SBUF. Copies
            alternate ScalarE/VectorE so neither queue clogs."""
            pa = alloc_group()
            for n in range(2):
                for k in range(DIN // P):
                    lhsT = Wsb[j][:, k, n * P:(n + 1) * P]
                    for bb in range(NB):
                        mm(pa[n][bb], lhsT, k, bb, k == 0, k == DIN // P - 1)
            for n in range(2):
                for bb in range(NB):
                    dst = HACC[j][:, n, bb * 512:(bb + 1) * 512]
                    if (n * NB + bb) % 2 == 0:
                        nc.scalar.activation(dst, pa[n][bb][:, :], AF.Copy)
                    else:
                        nc.vector.tensor_copy(dst, pa[n][bb][:, :])

        def early_group(j, psums):
            """AR-independent part of stage j's contraction."""
            pre = DIN // P if j in PARKED else 0
            for n in range(2):
                if pre:
                    for bb in range(NB):
                        nc.tensor.matmul(
                            psums[n][bb][:, :], IDT[:, :],
                            HACC[j][:, n, bb * 512:(bb + 1) * 512],
                            start=True, stop=False)
                for k in range(pre, KO[j] - 2):
                    lhsT = Wsb[j][:, k, n * P:(n + 1) * P]
                    for bb in range(NB):
                        mm(psums[n][bb], lhsT, k, bb,
                           pre == 0 and k == 0, False)

        def late_group(j, psums):
            # chunk-major order: psum (n, bb) groups complete progressively
            # so the relu/stats pipeline starts before the last matmul
            for bb in range(NB):
                for n in range(2):
                    for k in (KO[j] - 2, KO[j] - 1):
                        lhsT = Wsb[j][:, k, n * P:(n + 1) * P]
                        mm(psums[n][bb], lhsT, k, bb, False, k == KO[j] - 1)

        def out_group(ks, first):
            pso = [ps.tile([P, 512], F32, tag="pt", name="pt")
                   for _ in range(NB)]
            for k in ks:
                lhsT = WO[:, k, :]
                for bb in range(NB):
                    nc.tensor.matmul(
                        pso[bb][:, :], lhsT,
                        F[k][:, bb * 512:(bb + 1) * 512],
                        start=(k == ks[0]), stop=(k == ks[-1]))
            for bb in range(NB):
                dst = OUTACC[:, bb * 512:(bb + 1) * 512]
                if first:
                    nc.vector.tensor_scalar_add(dst, pso[bb][:, :],
                                                BOUT[:, 0:1])
                else:
                    nc.vector.tensor_add(dst, dst, pso[bb][:, :])

        # stage 0: everything available immediately
        psums = alloc_group()
        early_group(0, psums)
        late_group(0, psums)

        for i in range(K):
            # ---- relu + bias: PSUM -> bf16 h in SBUF ----
            # n=0 chunks on ScalarE (relu/square with accum_out), n=1 on
            # VectorE (relu + bn_stats), in psum-completion order so both
            # pipelines drain right behind the last matmul.
            hs = [hp.tile([P, BSH], BF, tag=f"h{n}", name=f"h{n}") for n in range(2)]
            sums = small.tile([P, NB], F32, tag="sums")
            sqs = small.tile([P, NB], F32, tag="sqs")
            st = small.tile([P, NB, 6], F32, tag="st")
            mv1 = small.tile([P, 2], F32, tag="mv1")
            arin = small.tile([P, 2, 2], F32, tag="arin")
            for bb in range(NB):
                c0 = hs[0][:, bb * 512:(bb + 1) * 512]
                c1 = hs[1][:, bb * 512:(bb + 1) * 512]
                nc.scalar.activation(
                    c0, psums[0][bb][:, :], AF.Relu,
                    bias=BV[:, i, 0:1], scale=1.0,
                    accum_out=sums[:, bb:bb + 1],
                )
                # n=0 sum-of-squares: first chunks on ScalarE (accum_out),
                # later chunks via GpSimd square + VectorE reduce, so no
                # single engine's queue becomes the drain bottleneck.
                if bb < 2:
                    scr = scrp.tile([P, 512], BF, tag="scr", name="scr")
                    nc.scalar.activation(
                        scr[:, :], c0, AF.Square,
                        accum_out=sqs[:, bb:bb + 1])
                else:
                    scr = scrp.tile([P, 512], F32, tag="scrf", name="scrf")
                    nc.gpsimd.tensor_mul(scr[:, :], c0, c0)
                    nc.vector.tensor_reduce(
                        sqs[:, bb:bb + 1], scr[:, :],
                        axis=mybir.AxisListType.X, op=OP.add)
                nc.vector.tensor_scalar(
                    c1, psums[1][bb][:, :], BV[:, i, 1:2], 0.0,
                    op0=OP.add, op1=OP.max,
                )
                nc.vector.bn_stats(st[:, bb, :], c1)
            nc.vector.bn_aggr(mv1[:], st[:, :, :])
            # n=1: (mean, var) -> raw sums;  n=0: reduce the chunk sums
            nc.vector.tensor_scalar(
                arin[:, 1, 1:2], mv1[:, 0:1], mv1[:, 0:1], mv1[:, 1:2],
                op0=OP.mult, op1=OP.add)
            nc.vector.tensor_scalar_mul(arin[:, 1, 1:2], arin[:, 1, 1:2],
                                        float(BSH))
            nc.vector.tensor_scalar_mul(arin[:, 1, 0:1], mv1[:, 0:1],
                                        float(BSH))
            nc.vector.tensor_reduce(
                arin[:, 0, 0:1], sums[:, :], axis=mybir.AxisListType.X,
                op=OP.add)
            nc.vector.tensor_reduce(
                arin[:, 0, 1:2], sqs[:, :], axis=mybir.AxisListType.X,
                op=OP.add)

            # ---- cross-core exchange of the stats (2KB) ----
            ccin = dram.tile([P, 2, 2], F32, tag="ccin")
            ccout = dram.tile([N_CORES, P, 2, 2], F32, tag="ccout",
                              addr_space="Shared")
            nc.sync.dma_start(ccin[:], arin[:])
            nc.gpsimd.collective_compute(
                "AllGather", OP.bypass, replica_groups=groups,
                ins=[ccin.opt()], outs=[ccout.opt()],
            )

            # ---- AR-independent window fill (tensor queue) ----
            # two output-matmul k-tiles per window (features of stage i-1;
            # x k-tiles in window 0), whose psum banks recycle group_i's
            # drained chunks, then stage i+1's early contraction.
            if i == 0:
                out_group(range(4), True)
                for j in PARKED:
                    park_group(j)
            elif i == 6:
                pass            # w6 k-tiles deferred to w7 (short fill there)
            elif i == 7:
                out_group((14, 15, 16, 17), False)
            else:
                out_group((2 + 2 * i, 3 + 2 * i), False)
            if i < K - 1:
                psums = alloc_group()
                early_group(i + 1, psums)

            # ---- local 8-way sum of the gathered stats ----
            ag = small.tile([P, N_CORES, 2, 2], F32, tag="ag")
            nc.sync.dma_start(ag[:], ccout[:, :, :, :].rearrange(
                "r p a b -> p r a b"))
            ared = small.tile([P, 2, 2], F32, tag="ared")
            nc.vector.tensor_reduce(
                ared[:, :, :], ag[:, :, :, :].rearrange("p r a b -> p a b r"),
                axis=mybir.AxisListType.X, op=OP.add)

            # ---- BN affine coefficients from global stats ----
            mue = small.tile([P, 2, 2], F32, tag="mue")
            nv = small.tile([P, 2], F32, tag="nv")
            rstd = small.tile([P, 2], F32, tag="rstd")
            a_ = small.tile([P, 2], F32, tag="a_")
            cb = small.tile([P, 2], F32, tag="cb")
            nc.vector.tensor_scalar_mul(mue[:], ared[:, :, :], 1.0 / B)
            mu = mue[:, :, 0]
            nc.vector.tensor_mul(nv[:], mu, mu)
            nc.vector.tensor_sub(nv[:], nv[:], mue[:, :, 1])   # mu^2 - E2 = -var
            nc.scalar.activation(rstd[:], nv[:], AF.Sqrt,
                                 bias=EPSC[:, 0:1], scale=-1.0)  # sqrt(var+eps)
            nc.vector.reciprocal(rstd[:], rstd[:])
            nc.vector.tensor_mul(a_[:], GV[:, i, :], rstd[:])
            nc.vector.tensor_mul(cb[:], mu, a_[:])
            nc.vector.tensor_sub(cb[:], BEV[:, i, :], cb[:])   # beta - a*mu

            # ---- normalize into the F blocks (bf16): n=0 on ScalarE,
            # n=1 on VectorE, chunk-paired so the first gated matmul
            # starts ~0.5us after the coefficients ----
            for q in range(NB):
                sl = slice(q * 512, (q + 1) * 512)
                nc.scalar.activation(
                    F[DIN // P + 2 * i][:, sl], hs[0][:, sl],
                    AF.Identity, bias=cb[:, 0:1], scale=a_[:, 0:1])
                nc.vector.tensor_scalar(
                    F[DIN // P + 2 * i + 1][:, sl], hs[1][:, sl],
                    a_[:, 1:2], cb[:, 1:2],
                    op0=OP.mult, op1=OP.add,
                )

            # ---- gated (late) matmuls of the next stage ----
            if i < K - 1:
                late_group(i + 1, psums)

        # ---- epilogue: last two output k-tiles, chunk-progressive so
        # each chunk's add + store starts right behind its matmuls ----
        pso = [ps.tile([P, 512], F32, tag="pt", name="pt")
               for _ in range(NB)]
        for bb in range(NB):
            for k in (T_TOT - 2, T_TOT - 1):
                nc.tensor.matmul(
                    pso[bb][:, :], WO[:, k, :],
                    F[k][:, bb * 512:(bb + 1) * 512],
                    start=(k == T_TOT - 2), stop=(k == T_TOT - 1))
            dst = OUTACC[:, bb * 512:(bb + 1) * 512]
            nc.vector.tensor_add(dst, dst, pso[bb][:, :])
            nc.sync.dma_start(outT_d[:, bb * 512:(bb + 1) * 512], dst)


def _get_nc():
    if "nc" not in _NC_CACHE:
        _NC_CACHE["nc"] = _build_nc()
    return _NC_CACHE["nc"]


def kernel(x, W0, W1, W2, W3, W4, W5, W6, W7, b, gamma, beta, Wout, bout):
    Ws = [W0, W1, W2, W3, W4, W5, W6, W7]
    nc = _get_nc()

    def pack_vec(v):  # [8,256] -> [128, 8, 2]
        return np.ascontiguousarray(
            np.asarray(v, np.float32).reshape(K, 2, P).transpose(2, 0, 1))

    common = {}
    for i, W in enumerate(Ws):
        wt = np.asarray(W, np.float32).T.astype(bfloat16)        # [d_i, 256]
        common[f"w{i}"] = np.ascontiguousarray(
            wt.reshape(KO[i], P, WS).transpose(1, 0, 2))         # [128, ko, 256]
    wot = np.asarray(Wout, np.float32).T.astype(bfloat16)        # [2560, 128]
    common["wo"] = np.ascontiguousarray(
        wot.reshape(T_TOT, P, DOUT).transpose(1, 0, 2))          # [128, 20, 128]
    common["bv"] = pack_vec(b)
    common["gv"] = pack_vec(gamma)
    common["bev"] = pack_vec(beta)
    common["boutv"] = np.ascontiguousarray(
        np.asarray(bout, np.float32).reshape(P, 1))

    in_maps = []
    for c in range(N_CORES):
        xs = np.asarray(x[c * BSH:(c + 1) * BSH], np.float32)    # [2048, 512]
        xt = xs.T.astype(bfloat16)                               # [512, 2048]
        in_maps.append({
            **common,
            "xt": np.ascontiguousarray(
                xt.reshape(DIN // P, P, BSH).transpose(1, 0, 2)),
        })

    kw = {}
    if TRACE_DIR is not None:
        kw = dict(trace=True, tmpdir=TRACE_DIR)
    try:
        res = run_bass_kernel_spmd(nc, in_maps, list(range(N_CORES)), **kw)
    except Exception:
        # transient PJRT INTERNAL errors have been observed; retry once
        res = run_bass_kernel_spmd(nc, in_maps, list(range(N_CORES)), **kw)
    global LAST_RESULTS
    LAST_RESULTS = res
    out = np.empty((B, DOUT), np.float32)
    for c in range(N_CORES):
        out[c * BSH:(c + 1) * BSH] = res.results[c]["outT"].T
    return out
